# revision 36
# baseline (speedup 1.0000x reference)
"""ARD-RBF kernel matrix on 8 TRN2 NeuronCores.

Math (reference):
    alpha = softmax(alpha_raw^2)            (D,)
    var   = variance_raw^2                  scalar
    sq_ij = sum_d alpha_d (x1_id - x2_jd)^2
    out   = var * exp(-0.5 * sq)            (N, M) f32

Device formulation (rows of x1 sharded 8 ways; per core):
    out_ij = exp( cross_ij - 0.5*ra_i + ln var ) * exp(-0.5*rb_j)
    cross  = x1 @ (alpha * x2)^T            fp16 matmul, f32 PSUM accum

The pipeline is paced by ScalarE (ACT), the only engine that can do exp:
8.4M exps/core at 1 elem/lane/cycle @1.2GHz = ~61us with per-op overhead.
Everything else hides behind it:
  PE      54.6us at full clock (2.4GHz) -- has slack, but only at full
          clock; PE downclocks to 1.2GHz after stalls, so the schedule
          must never starve it (see group order below).
  DVE     bf16 column-scale by exp(-0.5 rb): ~39us.
  DMA     16MB out + 4.5MB in at ~358GB/s/core: ~57us aggregate.

Group order is COLUMN-major (g outer, t inner): the first 8 groups all
read x2 column-group 0, so only bia+x1+1MB of x2 gate the start of the
pipe (vs all 4.5MB with row-major order).  x2 column groups g1..g3
stream in behind.  exp(-0.5 rb) is DMA'd as one 16KB row and replicated
to 128 partitions by GpSimd partition_broadcast (2048 cols per chunk),
off every critical path.

Per group G = g*8 + t: PE 8 fp16 matmuls (2 k-tiles x 4 x 512 cols) into
a [128,2048] f32 PSUM half; ACT Exp w/ per-partition bias (-0.5 ra + ln
var) -> bf16 ot[G%4]; DVE multiplies by erb[g] slice; sync issues the
512KB output chunk.  2 PSUM halves, 4 ot slots.  The last group's
mul+DMA are split in two to shorten the tail.
"""

import math
import sys

import numpy as np

import ml_dtypes  # noqa: F401  (np bf16 dtype for device results)

if "/opt/trn_rl_repo" not in sys.path:
    sys.path.insert(0, "/opt/trn_rl_repo")

N, M, D = 8192, 8192, 256
NCORES = 8
NS = N // NCORES          # 1024 rows of x1 per core
P = 128                   # partitions
KT = D // P               # 2 k-tiles
NG = 4                    # x2 column groups
JG = M // NG              # 2048 cols per group
NJ = 512                  # matmul moving free dim (1 PSUM bank)
NT = NS // P              # 8 row tiles per core
NGRP = NG * NT            # 32 groups

_F16 = np.float16
_BF16 = ml_dtypes.bfloat16

_compiled = None

# tunables
WARM = 12                 # junk warmup matmuls to ramp PE p-state


def _build():
    import concourse.bass as bass
    import concourse.mybir as mybir
    from concourse.env import get_walrus_max_sem_num
    from contextlib import ExitStack

    # Shrink the kernel semaphore pool: the module epilogue emits per-sem
    # reset ops over the whole pool (~7us for the default ~200-sem range);
    # this kernel uses <30, so a 48-wide pool cuts the epilogue cost.
    base = get_walrus_max_sem_num()
    bass.get_kernel_semaphore_range = lambda: range(base, base + 48)

    dt = mybir.dt
    nc = bass.Bass()

    x1d = nc.declare_dram_parameter("x1d", [P, KT * NS], dt.float16, isOutput=False)
    x2d = nc.declare_dram_parameter("x2d", [P, NG * KT * JG], dt.float16, isOutput=False)
    rbd = nc.declare_dram_parameter("rbd", [P, M], dt.bfloat16, isOutput=False)
    biad = nc.declare_dram_parameter("biad", [P, NT], dt.float32, isOutput=False)
    outd = nc.declare_dram_parameter("out", [NS, M], dt.bfloat16, isOutput=True)

    exp_f = mybir.ActivationFunctionType.Exp
    njc = JG // NJ            # 4 matmul column chunks per group

    with ExitStack() as _ctx:
        ec = _ctx.enter_context
        x1s = ec(nc.sbuf_tensor("x1s", [P, KT * NS], dt.float16))
        x2s = ec(nc.sbuf_tensor("x2s", [P, NG * KT * JG], dt.float16))
        bis = ec(nc.sbuf_tensor("bis", [P, NT], dt.float32))
        erb = ec(nc.sbuf_tensor("erb", [P, M], dt.bfloat16))
        ot0 = ec(nc.sbuf_tensor("ot0", [P, JG], dt.bfloat16))
        ot1 = ec(nc.sbuf_tensor("ot1", [P, JG], dt.bfloat16))
        ot2 = ec(nc.sbuf_tensor("ot2", [P, JG], dt.bfloat16))
        ot3 = ec(nc.sbuf_tensor("ot3", [P, JG], dt.bfloat16))
        wrm = ec(nc.sbuf_tensor("wrm", [P, P + NJ], dt.float16))  # uninit junk
        scr = ec(nc.sbuf_tensor("scr", [1, 32], dt.float32))      # table preload
        ps0 = ec(nc.psum_tensor("ps0", [P, JG], dt.float32))
        ps1 = ec(nc.psum_tensor("ps1", [P, JG], dt.float32))
        # DMA-completion sems: the 16 SDMA engines inc independently, so only
        # FULL-count waits on a sem are race-free.
        ot4 = ec(nc.sbuf_tensor("ot4", [P, JG], dt.bfloat16))
        ot5 = ec(nc.sbuf_tensor("ot5", [P, JG], dt.bfloat16))
        s_in = ec(nc.semaphore("s_in"))    # bia + x1 k0 + warm (full = 48)
        s_x1b = ec(nc.semaphore("s_x1b"))  # x1 k1              (full = 16)
        s_x2a = ec(nc.semaphore("s_x2a"))  # x2 g0 k0 cols 0-512   (full = 16)
        s_x2b = ec(nc.semaphore("s_x2b"))  # x2 g0 k1 cols 0-512   (full = 16)
        s_x2a2 = ec(nc.semaphore("s_x2a2"))  # x2 g0 k0 cols 512-2048 (full = 16)
        s_x2b2 = ec(nc.semaphore("s_x2b2"))  # x2 g0 k1 cols 512-2048 (full = 16)
        s_x2c = ec(nc.semaphore("s_x2c"))  # x2 g1              (full = 16)
        s_x2f = ec(nc.semaphore("s_x2f"))  # x2 g2 + g3         (full = 32)
        ebA = ec(nc.semaphore("ebA"))      # erb cols g0        (full = 16)
        ebB = ec(nc.semaphore("ebB"))      # erb cols g1        (full = 16)
        ebC = ec(nc.semaphore("ebC"))      # erb cols g2+g3     (full = 32)
        dp0 = ec(nc.semaphore("dp0"))      # out chunks per ot slot
        dp1 = ec(nc.semaphore("dp1"))
        dp2 = ec(nc.semaphore("dp2"))
        dp3 = ec(nc.semaphore("dp3"))
        dp4 = ec(nc.semaphore("dp4"))
        dp5 = ec(nc.semaphore("dp5"))
        pes = ec(nc.semaphore("pes"))
        acs = ec(nc.semaphore("acs"))
        vcs = ec(nc.semaphore("vcs"))
        block = ec(nc.Block())
        ots = [ot0, ot1, ot2, ot3, ot4, ot5]
        OTN = len(ots)
        pss = [ps0, ps1]
        dps = [dp0, dp1, dp2, dp3, dp4, dp5]
        ebs_l = [ebA, ebB, ebC, ebC]
        ebs_n = [16, 16, 16, 16]

        def gt(G):
            g, t = divmod(G, NT)
            return g, t

        # unit bookkeeping: G0's PE/ACT work is split into 4 x 512-col chunks
        # (starts the exp stream ~6us earlier); the last two groups' mul+DMA
        # are split in half to drain the tail faster.
        TAIL = (NGRP - 2, NGRP - 1)
        def dve_units(G):
            if G in TAIL:
                h = JG // 2
                return [(0, h), (h, JG)]
            return [(0, JG)]
        # cumulative counts after group G completes
        pes_after = lambda G: 4 + G            # G0 contributes 4
        acs_after = lambda G: 4 + G
        vcs_after = {}
        c = 0
        for G in range(NGRP):
            c += len(dve_units(G))
            vcs_after[G] = c

        @block.sync
        def _(sync):
            # warm up the output DMA queue (ring-init latency) with a junk read
            sync.dma_start(scr[0:1, 0:8], biad[0:1, 0:NT]).then_inc(s_in, 16)
            # output chunks only; inputs are issued from the gpsimd queue
            for G in range(NGRP):
                g, t = gt(G)
                us = dve_units(G)
                for n, (lo, hi) in enumerate(us):
                    sync.wait_ge(vcs, vcs_after[G] - len(us) + 1 + n)
                    sync.dma_start(
                        outd[t * P:(t + 1) * P, g * JG + lo:g * JG + hi],
                        ots[G % OTN][:, lo:hi],
                    ).then_inc(dps[G % OTN], 16)
            for s in range(OTN):
                chunks = sum(len(dve_units(G)) for G in range(NGRP) if G % OTN == s)
                sync.wait_ge(dps[s], 16 * chunks)

        @block.gpsimd
        def _(gpsimd):
            # single input queue, strict priority order: items land in the
            # order the pipeline first needs them (k0 path first so PE's
            # first half-group can start before the k1 data arrives)
            gpsimd.dma_start(bis[:, :], biad[:, :]).then_inc(s_in, 16)
            gpsimd.dma_start(x1s[:, 0:NS], x1d[:, 0:NS]).then_inc(s_in, 16)
            gpsimd.dma_start(x2s[:, 0:NJ], x2d[:, 0:NJ]).then_inc(s_x2a, 16)
            gpsimd.dma_start(x1s[:, NS:2 * NS], x1d[:, NS:2 * NS]).then_inc(s_x1b, 16)
            gpsimd.dma_start(x2s[:, JG:JG + NJ], x2d[:, JG:JG + NJ]).then_inc(s_x2b, 16)
            gpsimd.dma_start(x2s[:, NJ:JG], x2d[:, NJ:JG]).then_inc(s_x2a2, 16)
            gpsimd.dma_start(x2s[:, JG + NJ:2 * JG], x2d[:, JG + NJ:2 * JG]).then_inc(s_x2b2, 16)
            gpsimd.dma_start(erb[:, 0:JG], rbd[:, 0:JG]).then_inc(ebA, 16)
            gpsimd.dma_start(erb[:, JG:2 * JG], rbd[:, JG:2 * JG]).then_inc(ebB, 16)
            gpsimd.dma_start(x2s[:, 2 * JG:4 * JG], x2d[:, 2 * JG:4 * JG]).then_inc(s_x2c, 16)
            gpsimd.dma_start(erb[:, 2 * JG:4 * JG], rbd[:, 2 * JG:4 * JG]).then_inc(ebC, 16)
            gpsimd.dma_start(x2s[:, 4 * JG:6 * JG], x2d[:, 4 * JG:6 * JG]).then_inc(s_x2f, 16)
            gpsimd.dma_start(x2s[:, 6 * JG:8 * JG], x2d[:, 6 * JG:8 * JG]).then_inc(s_x2f, 16)

        @block.tensor
        def _(tensor):
            # PE warm-up on junk data while inputs stream in (p-state ramp)
            for _ in range(WARM):
                tensor.matmul(ps0[:, 0:NJ], wrm[:, 0:P], wrm[:, P:P + NJ],
                              start=True, stop=True)
            # G0: j-outer, k-inner 512-col chunk pairs, each completing a PSUM
            # bank that ACT can drain immediately (pes inc per chunk)
            for j in range(njc):
                for k in range(KT):
                    if j == 0 and k == 0:
                        tensor.wait_ge(s_in, 48)     # bia + x1 k0 (+warm)
                        tensor.wait_ge(s_x2a, 16)    # x2 g0 k0 cols 0-512
                    if j == 0 and k == 1:
                        tensor.wait_ge(s_x1b, 16)
                        tensor.wait_ge(s_x2b, 16)
                    if j == 1 and k == 0:
                        tensor.wait_ge(s_x2a2, 16)   # k0 cols 512-2048
                    if j == 1 and k == 1:
                        tensor.wait_ge(s_x2b2, 16)
                    mm = tensor.matmul(
                        ps0[:, j * NJ:(j + 1) * NJ],
                        x1s[:, k * NS: k * NS + P],
                        x2s[:, k * JG + j * NJ: k * JG + (j + 1) * NJ],
                        start=(k == 0),
                        stop=(k == KT - 1),
                    )
                mm.then_inc(pes)
            for G in range(1, NGRP):
                g, t = gt(G)
                if G == NT:
                    tensor.wait_ge(s_x2c, 16)    # entering column group 1
                if G == 2 * NT:
                    tensor.wait_ge(s_x2f, 32)    # g2 + g3
                if G >= 2:
                    tensor.wait_ge(acs, acs_after(G - 2))  # psum half free
                ps = pss[G % 2]
                for k in range(KT):
                    for j in range(njc):
                        mm = tensor.matmul(
                            ps[:, j * NJ:(j + 1) * NJ],
                            x1s[:, k * NS + t * P: k * NS + (t + 1) * P],
                            x2s[:, (g * KT + k) * JG + j * NJ:
                                   (g * KT + k) * JG + (j + 1) * NJ],
                            start=(k == 0),
                            stop=(k == KT - 1),
                        )
                mm.then_inc(pes)

        @block.scalar
        def _(scalar):
            # touch Exp early so ACT_TABLE_LOAD overlaps the input DMAs
            scalar.activation(scr[0:1, 16:32], scr[0:1, 0:16], exp_f)
            scalar.wait_ge(s_in, 48)
            # G0: drain each 512-col PSUM bank as PE finishes it
            for c in range(njc):
                scalar.wait_ge(pes, c + 1)
                scalar.activation(
                    ots[0][:, c * NJ:(c + 1) * NJ],
                    ps0[:, c * NJ:(c + 1) * NJ],
                    exp_f,
                    bias=bis[:, 0:1],
                    scale=1.0,
                ).then_inc(acs)
            for G in range(1, NGRP):
                g, t = gt(G)
                scalar.wait_ge(pes, pes_after(G))
                if G >= OTN:
                    scalar.wait_ge(dps[G % OTN], 16 * (G // OTN))  # ot slot free
                scalar.activation(
                    ots[G % OTN][:, :],
                    pss[G % 2][:, :],
                    exp_f,
                    bias=bis[:, t:t + 1],
                    scale=1.0,
                ).then_inc(acs)

        @block.vector
        def _(vector):
            for G in range(NGRP):
                g, t = gt(G)
                vector.wait_ge(acs, acs_after(G))
                if t == 0:
                    vector.wait_ge(ebs_l[g], ebs_n[g])
                for lo, hi in dve_units(G):
                    vector.tensor_mul(ots[G % OTN][:, lo:hi], ots[G % OTN][:, lo:hi],
                                      erb[:, g * JG + lo:g * JG + hi]).then_inc(vcs)

    return nc


def _prep(x1, x2, alpha_raw, variance_raw):
    x1 = np.ascontiguousarray(np.asarray(x1, dtype=np.float32))
    x2 = np.ascontiguousarray(np.asarray(x2, dtype=np.float32))
    ar = np.asarray(alpha_raw, dtype=np.float64).reshape(-1)
    vr = np.asarray(variance_raw, dtype=np.float64).reshape(-1)

    a2 = ar * ar
    e = np.exp(a2 - a2.max())
    alpha = e / e.sum()                                   # (D,) f64
    var = float(vr[0]) ** 2
    if var > 0.0:
        logvar, post = math.log(var), None
    else:
        logvar, post = 0.0, var

    b = alpha[None, :] * x2.astype(np.float64)            # (M, D)
    x2tm = b.T.reshape(KT, P, M).astype(_F16)             # [k, p, col]
    # device layout: col index = g*(KT*JG) + k*JG + j
    x2c = np.ascontiguousarray(
        x2tm.reshape(KT, P, NG, JG).transpose(1, 2, 0, 3).reshape(P, NG * KT * JG))
    x1tm = x1.T.reshape(KT, P, N).astype(_F16)            # [k, p, row]

    ra = (x1.astype(np.float64) ** 2) @ alpha             # (N,)
    rb = (x2.astype(np.float64) ** 2) @ alpha             # (M,)
    bia = (-0.5 * ra + logvar).astype(np.float32)         # (N,)
    rbrow = np.exp(-0.5 * rb).astype(_BF16).reshape(1, M)
    rbd = np.ascontiguousarray(np.broadcast_to(rbrow, (P, M)))

    in_maps = []
    for c in range(NCORES):
        sl = slice(c * NS, (c + 1) * NS)
        x1c = np.ascontiguousarray(
            np.concatenate([x1tm[0][:, sl], x1tm[1][:, sl]], axis=1))
        bia2 = np.ascontiguousarray(
            bia[sl].reshape(NT, P).T.astype(np.float32))   # [p, t]
        in_maps.append({
            "x1d": x1c,
            "x2d": x2c,
            "rbd": rbd,
            "biad": bia2,
        })
    return in_maps, post


def _run(in_maps, trace=False):
    global _compiled
    from concourse.bass_utils import run_bass_kernel_spmd

    if _compiled is None:
        _compiled = _build()
    return run_bass_kernel_spmd(
        _compiled, in_maps, core_ids=list(range(NCORES)), trace=trace
    )


def kernel(x1, x2, alpha_raw, variance_raw):
    in_maps, post = _prep(x1, x2, alpha_raw, variance_raw)
    res = _run(in_maps)
    full = np.concatenate(
        [np.asarray(res.results[c]["out"]).astype(np.float32) for c in range(NCORES)],
        axis=0)
    if post is not None:
        full = (full * post).astype(np.float32)
    return full
